# revision 13
# baseline (speedup 1.0000x reference)
"""Bass/Tile attention kernel for Trainium2, SPMD over 8 NeuronCores.

Problem: B,S,D,DK = 8,2048,512,64 full bidirectional attention with
softmax; returns (attended[B,S,DK], weights[B,S,S]).

Sharding: data-parallel over batch — core b handles batch b. No
collectives needed. W_q/W_k/W_v replicated.

Per-core dataflow (S=2048 seq, D=512 model, DK=64 head), bf16 compute:
  x[2048,512] --cast+DMA-xbar-transpose--> xT[512,2048] bf16
  qT/kT[64,2048] = W.T @ xT ; vT --PE transpose--> v[2048,64]
  loop over 16 query tiles t:         (weights output path)
    S_t[128,2048] = qT_t.T @ kT
    expS_t = exp(0.125*S_t)  (ACT, accum_out -> Z_t row sums)
    P_t = expS_t * (1/Z_t)   (DVE) --> DMA out weights tile (streams early)
  loop over 16 key tiles u:           (attended path)
    ST_u[128,2048] = kT_u.T @ qT ; expST_u = exp(0.125*ST_u)
    att[sq,64] += expST_u[:,sq_tile].T @ v_u   (PSUM accum over u)
  att_t = att_t * (1/Z_t) --> DMA out
"""

import numpy as np

B, S, D, DK = 8, 2048, 512, 64
P = 128          # partition size
SQT = S // P     # 16 query tiles
C = D // P       # 4 d-chunks
U = S // P       # 16 key tiles
NCH = S // 512   # 4 free-dim 512-chunks

_CACHE = {}


def _build():
    from concourse import bacc, mybir, tile
    from concourse.masks import make_identity

    f32 = mybir.dt.float32
    bf16 = mybir.dt.bfloat16
    Exp = mybir.ActivationFunctionType.Exp

    nc = bacc.Bacc("TRN2", target_bir_lowering=False, debug=False,
                   num_devices=B)

    x_d = nc.dram_tensor("x", [S, D], f32, kind="ExternalInput").ap()
    wq_d = nc.dram_tensor("wq", [D, DK], f32, kind="ExternalInput").ap()
    wk_d = nc.dram_tensor("wk", [D, DK], f32, kind="ExternalInput").ap()
    wv_d = nc.dram_tensor("wv", [D, DK], f32, kind="ExternalInput").ap()
    att_d = nc.dram_tensor("att", [S, DK], f32, kind="ExternalOutput").ap()
    wts_d = nc.dram_tensor("wts", [S, S], f32, kind="ExternalOutput").ap()

    with tile.TileContext(nc) as tc:
        with (
            tc.tile_pool(name="const", bufs=1) as const,
            tc.tile_pool(name="xf", bufs=3) as xfpool,
            tc.tile_pool(name="expp", bufs=2) as expp,
            tc.tile_pool(name="pout", bufs=3) as pout,
            tc.tile_pool(name="spsum", bufs=1, space="PSUM") as spsum,
            tc.tile_pool(name="stpsum", bufs=1, space="PSUM") as stpsum,
            tc.tile_pool(name="attpsum", bufs=1, space="PSUM") as attpsum,
        ):
            ident_b = const.tile([P, P], bf16)
            make_identity(nc, ident_b)

            # --- weights: [512,64] f32 -> [128, 3, c, 64] bf16 ---
            wstage = xfpool.tile([P, 3, C, DK], f32, tag="wstage")
            for i, w_d in enumerate((wq_d, wk_d, wv_d)):
                nc.sync.dma_start(
                    wstage[:, i], w_d.rearrange("(c p) k -> p c k", p=P))
            w_sb3 = const.tile([P, 3, C, DK], bf16)
            nc.vector.tensor_copy(w_sb3[:], wstage[:])
            wq_sb = w_sb3[:, 0]
            wk_sb = w_sb3[:, 1]
            wv_sb = w_sb3[:, 2]

            # --- x load + cast + PE transpose -> xT [128, c, 2048] ---
            xT = const.tile([P, C, S], bf16)
            for t in range(SQT):
                xf = xfpool.tile([P, D], f32, tag="xf")
                nc.sync.dma_start(xf[:], x_d[t * P:(t + 1) * P, :])
                xb = xfpool.tile([P, D], bf16, tag="xb")
                nc.vector.tensor_copy(xb[:], xf[:])
                trp = stpsum.tile([P, 512], bf16, tag="st")
                for c in range(C):
                    nc.tensor.transpose(
                        trp[:, c * P:(c + 1) * P], xb[:, c * P:(c + 1) * P],
                        ident_b[:])
                nc.vector.tensor_copy(
                    xT[:, :, t * P:(t + 1) * P],
                    trp[:].rearrange("p (c f) -> p c f", c=C))

            # --- qT, kT [64, 2048] bf16 ; vT -> v [128, u*64] bf16 ---
            qT = const.tile([DK, S], bf16)
            kT = const.tile([DK, S], bf16)
            vT = const.tile([DK, S], bf16)
            for n in range(NCH):
                sl = slice(n * 512, (n + 1) * 512)
                for (w_sb, dst) in ((wq_sb, qT), (wk_sb, kT), (wv_sb, vT)):
                    mm = stpsum.tile([DK, 512], f32, tag="st")
                    for c in range(C):
                        nc.tensor.matmul(
                            mm[:], w_sb[:, c, :], xT[:, c, sl],
                            start=(c == 0), stop=(c == C - 1))
                    nc.vector.tensor_copy(dst[:, sl], mm[:])

            v_sb = const.tile([P, U * DK], bf16)
            for g in range(2):
                trp = stpsum.tile([P, 512], bf16, tag="st")
                for j in range(8):
                    u = g * 8 + j
                    nc.tensor.transpose(
                        trp[:, j * DK:(j + 1) * DK],
                        vT[:, u * P:(u + 1) * P], ident_b[:DK, :DK])
                nc.vector.tensor_copy(v_sb[:, g * 512:(g + 1) * 512], trp[:])

            recip = const.tile([P, SQT], f32)  # 1/Z per query tile

            # --- main loop: weights path (t = query tile) interleaved
            # --- with attended path (t = key tile); they are independent,
            # --- which keeps PE/ACT/DVE/DMA all fed concurrently.
            attp = attpsum.tile([P, SQT * DK], f32)
            for t in range(SQT):
                tsl = slice(t * P, (t + 1) * P)

                # - weights path: S_t = q_t k^T, softmax row-normalize -
                sp = spsum.tile([P, S], f32, tag="s")
                for n in range(NCH):
                    nc.tensor.matmul(
                        sp[:, n * 512:(n + 1) * 512],
                        qT[:, tsl], kT[:, n * 512:(n + 1) * 512],
                        start=True, stop=True)
                expS = expp.tile([P, S], bf16, tag="exp")
                zs = expp.tile([P, 1], f32, tag="zs")
                nc.scalar.activation(expS[:], sp[:], Exp,
                                     scale=0.125, accum_out=zs[:])
                nc.vector.reciprocal(recip[:, t:t + 1], zs[:])
                p_t = pout.tile([P, S], f32, tag="p")
                nc.vector.tensor_scalar_mul(p_t[:], expS[:], recip[:, t:t + 1])
                nc.sync.dma_start(wts_d[tsl, :], p_t[:])

                # - attended path: ST_t = k_t q^T, attp += expST^T v_t -
                expST = expp.tile([P, S], bf16, tag="expst")
                for h in range(2):
                    stp = stpsum.tile([P, 1024], f32, tag="st")
                    for i in range(2):
                        n = h * 2 + i
                        nc.tensor.matmul(
                            stp[:, i * 512:(i + 1) * 512],
                            kT[:, tsl], qT[:, n * 512:(n + 1) * 512],
                            start=True, stop=True)
                    nc.scalar.activation(
                        expST[:, h * 1024:(h + 1) * 1024], stp[:], Exp,
                        scale=0.125)
                # attp is 2 PSUM banks (8 j-regions each); start/stop are
                # bank-granular: one start (zeroes the bank) and one stop
                # per bank per accumulation sweep.
                for j in range(SQT):
                    nc.tensor.matmul(
                        attp[:, j * DK:(j + 1) * DK],
                        expST[:, j * P:(j + 1) * P],
                        v_sb[:, t * DK:(t + 1) * DK],
                        start=(t == 0 and j % 8 == 0),
                        stop=(t == U - 1 and j % 8 == 7))

            # --- att = attp * recip -> out ---
            for j in range(SQT):
                a_j = pout.tile([P, DK], f32, tag="a")
                nc.vector.tensor_scalar_mul(
                    a_j[:], attp[:, j * DK:(j + 1) * DK], recip[:, j:j + 1])
                nc.sync.dma_start(att_d[j * P:(j + 1) * P, :], a_j[:])

    nc.compile()
    return nc


def get_nc():
    if "nc" not in _CACHE:
        _CACHE["nc"] = _build()
    return _CACHE["nc"]


def kernel(inputs, W_q, W_k, W_v):
    from concourse.bass_utils import run_bass_kernel_spmd

    nc = get_nc()
    inputs = np.ascontiguousarray(inputs, dtype=np.float32)
    in_maps = [
        {
            "x": inputs[b],
            "wq": np.ascontiguousarray(W_q, dtype=np.float32),
            "wk": np.ascontiguousarray(W_k, dtype=np.float32),
            "wv": np.ascontiguousarray(W_v, dtype=np.float32),
        }
        for b in range(B)
    ]
    res = run_bass_kernel_spmd(nc, in_maps, core_ids=list(range(B)))
    att = np.stack([res.results[b]["att"] for b in range(B)])
    wts = np.stack([res.results[b]["wts"] for b in range(B)])
    return att, wts


# revision 14
# speedup vs baseline: 1.1547x; 1.1547x over previous
"""Bass/Tile attention kernel for Trainium2, SPMD over 8 NeuronCores.

Problem: B,S,D,DK = 8,2048,512,64 full bidirectional attention with
softmax; returns (attended[B,S,DK], weights[B,S,S]).

Sharding: data-parallel over batch — core b handles batch b. No
collectives needed. W_q/W_k/W_v replicated.

Per-core dataflow (S=2048 seq, D=512 model, DK=64 head), bf16 compute:
  x[2048,512] --cast + PE transpose--> xT[512,2048] bf16
  qT/kT[64,2048] = W.T @ xT ; vT --PE transpose--> v[2048,64]
  main loop over 16 tiles t (both paths, software-pipelined):
    weights path (t = query tile):
      S_t[128,2048] = qT_t.T @ kT   (PSUM)
      expS_t = exp(0.125*S_t)       (ACT, accum_out -> Z_t row sums)
      P_t = expS_t * (1/Z_t)        (DVE, bf16) --> DMA out (bf16 wts)
    attended path (t = key tile):
      ST_t[128,2048] = kT_t.T @ qT ; expST_t = exp(0.125*ST_t)
      PV for tile t-1 (pipelined so PE never waits on this t's exp):
        att[sq,64] += expST_{t-1}[:,sq_tile].T @ v_{t-1}
  att_t = att_t * (1/Z_t) --> DMA out (f32)
"""

import numpy as np

B, S, D, DK = 8, 2048, 512, 64
P = 128          # partition size
SQT = S // P     # 16 query tiles
C = D // P       # 4 d-chunks
U = S // P       # 16 key tiles
NCH = S // 512   # 4 free-dim 512-chunks

_CACHE = {}


def _build():
    from concourse import bacc, mybir, tile
    from concourse.masks import make_identity

    f32 = mybir.dt.float32
    bf16 = mybir.dt.bfloat16
    Exp = mybir.ActivationFunctionType.Exp

    nc = bacc.Bacc("TRN2", target_bir_lowering=False, debug=False,
                   num_devices=B)

    x_d = nc.dram_tensor("x", [S, D], f32, kind="ExternalInput").ap()
    wq_d = nc.dram_tensor("wq", [D, DK], f32, kind="ExternalInput").ap()
    wk_d = nc.dram_tensor("wk", [D, DK], f32, kind="ExternalInput").ap()
    wv_d = nc.dram_tensor("wv", [D, DK], f32, kind="ExternalInput").ap()
    att_d = nc.dram_tensor("att", [S, DK], f32, kind="ExternalOutput").ap()
    wts_d = nc.dram_tensor("wts", [S, S], bf16, kind="ExternalOutput").ap()

    with tile.TileContext(nc) as tc:
        with tc.tile_pool(name="const", bufs=1) as const:
            ident_b = const.tile([P, P], bf16)
            make_identity(nc, ident_b)

            xT = const.tile([P, C, S], bf16)
            qT = const.tile([DK, S], bf16)
            kT = const.tile([DK, S], bf16)
            vT = const.tile([DK, S], bf16)
            v_sb = const.tile([P, U * DK], bf16)
            recip = const.tile([P, SQT], f32)   # 1/Z per query tile

            # ---------------- setup phase (own PSUM pools) ----------------
            with (
                tc.tile_pool(name="xstage", bufs=1) as xstage,
                tc.tile_pool(name="trps", bufs=3, space="PSUM") as trps,
                tc.tile_pool(name="qkvps", bufs=2, space="PSUM") as qkvps,
            ):
                # weights: [512,64] f32 -> [128, 3, c, 64] bf16
                wstage = xstage.tile([P, 3, C, DK], f32, tag="wstage")
                for i, w_d in enumerate((wq_d, wk_d, wv_d)):
                    nc.sync.dma_start(
                        wstage[:, i], w_d.rearrange("(c p) k -> p c k", p=P))
                w_sb3 = const.tile([P, 3, C, DK], bf16)
                nc.vector.tensor_copy(w_sb3[:], wstage[:])

                # x: one big DMA, one big cast, then PE transposes
                xf = xstage.tile([P, SQT, D], f32, tag="xf")
                nc.sync.dma_start(
                    xf[:], x_d.rearrange("(t p) d -> p t d", p=P))
                xb = xstage.tile([P, SQT, D], bf16, tag="xb")
                nc.vector.tensor_copy(xb[:], xf[:])
                for t in range(SQT):
                    trp = trps.tile([P, 512], bf16, tag="tr")
                    for c in range(C):
                        nc.tensor.transpose(
                            trp[:, c * P:(c + 1) * P],
                            xb[:, t, c * P:(c + 1) * P], ident_b[:])
                    nc.vector.tensor_copy(
                        xT[:, :, t * P:(t + 1) * P],
                        trp[:].rearrange("p (c f) -> p c f", c=C))

                # qT, kT, vT = W.T @ xT
                for n in range(NCH):
                    sl = slice(n * 512, (n + 1) * 512)
                    for w_i, dst in ((0, qT), (1, kT), (2, vT)):
                        mm = qkvps.tile([DK, 512], f32, tag="qkv")
                        for c in range(C):
                            nc.tensor.matmul(
                                mm[:], w_sb3[:, w_i, c, :], xT[:, c, sl],
                                start=(c == 0), stop=(c == C - 1))
                        nc.vector.tensor_copy(dst[:, sl], mm[:])

                # v natural [128, u*64] via PE transpose of vT
                for g in range(2):
                    trp = trps.tile([P, 512], bf16, tag="tr")
                    for j in range(8):
                        u = g * 8 + j
                        nc.tensor.transpose(
                            trp[:, j * DK:(j + 1) * DK],
                            vT[:, u * P:(u + 1) * P], ident_b[:DK, :DK])
                    nc.vector.tensor_copy(
                        v_sb[:, g * 512:(g + 1) * 512], trp[:])

            # ---------------- main loop (own PSUM pools) ----------------
            with (
                tc.tile_pool(name="expp", bufs=2) as expp,
                tc.tile_pool(name="pout", bufs=3) as pout,
                tc.tile_pool(name="spsum", bufs=1, space="PSUM") as spsum,
                tc.tile_pool(name="stpsum", bufs=1, space="PSUM") as stpsum,
                tc.tile_pool(name="attpsum", bufs=1, space="PSUM") as attps,
            ):
                attp = attps.tile([P, SQT * DK], f32)
                expST_prev = None

                def pv_step(t, expST):
                    # attp is 2 PSUM banks (8 j-regions each); start/stop
                    # are bank-granular: one start (zeroes the bank) and
                    # one stop per bank per accumulation sweep.
                    for j in range(SQT):
                        nc.tensor.matmul(
                            attp[:, j * DK:(j + 1) * DK],
                            expST[:, j * P:(j + 1) * P],
                            v_sb[:, t * DK:(t + 1) * DK],
                            start=(t == 0 and j % 8 == 0),
                            stop=(t == U - 1 and j % 8 == 7))

                for t in range(SQT):
                    tsl = slice(t * P, (t + 1) * P)

                    # - weights path -
                    sp = spsum.tile([P, S], f32, tag="s")
                    for n in range(NCH):
                        nc.tensor.matmul(
                            sp[:, n * 512:(n + 1) * 512],
                            qT[:, tsl], kT[:, n * 512:(n + 1) * 512],
                            start=True, stop=True)
                    expS = expp.tile([P, S], bf16, tag="exp")
                    zs = expp.tile([P, 1], f32, tag="zs")
                    nc.scalar.activation(expS[:], sp[:], Exp,
                                         scale=0.125, accum_out=zs[:])
                    nc.vector.reciprocal(recip[:, t:t + 1], zs[:])
                    p_t = pout.tile([P, S], bf16, tag="p")
                    nc.vector.tensor_scalar_mul(
                        p_t[:], expS[:], recip[:, t:t + 1])
                    nc.sync.dma_start(wts_d[tsl, :], p_t[:])

                    # - attended path: ST_t, exp; PV for t-1 -
                    expST = expp.tile([P, S], bf16, tag="expst")
                    for h in range(2):
                        stp = stpsum.tile([P, 1024], f32, tag="st")
                        for i in range(2):
                            n = h * 2 + i
                            nc.tensor.matmul(
                                stp[:, i * 512:(i + 1) * 512],
                                kT[:, tsl], qT[:, n * 512:(n + 1) * 512],
                                start=True, stop=True)
                        nc.scalar.activation(
                            expST[:, h * 1024:(h + 1) * 1024], stp[:], Exp,
                            scale=0.125)
                    if expST_prev is not None:
                        pv_step(t - 1, expST_prev)
                    expST_prev = expST
                pv_step(SQT - 1, expST_prev)

                # - att = attp * recip -> out -
                for j in range(SQT):
                    a_j = pout.tile([P, DK], f32, tag="a")
                    nc.vector.tensor_scalar_mul(
                        a_j[:], attp[:, j * DK:(j + 1) * DK],
                        recip[:, j:j + 1])
                    nc.sync.dma_start(att_d[j * P:(j + 1) * P, :], a_j[:])

    nc.compile()
    return nc


def get_nc():
    if "nc" not in _CACHE:
        _CACHE["nc"] = _build()
    return _CACHE["nc"]


def kernel(inputs, W_q, W_k, W_v):
    from concourse.bass_utils import run_bass_kernel_spmd

    nc = get_nc()
    inputs = np.ascontiguousarray(inputs, dtype=np.float32)
    in_maps = [
        {
            "x": inputs[b],
            "wq": np.ascontiguousarray(W_q, dtype=np.float32),
            "wk": np.ascontiguousarray(W_k, dtype=np.float32),
            "wv": np.ascontiguousarray(W_v, dtype=np.float32),
        }
        for b in range(B)
    ]
    res = run_bass_kernel_spmd(nc, in_maps, core_ids=list(range(B)))
    att = np.stack([res.results[b]["att"] for b in range(B)])
    wts = np.stack(
        [res.results[b]["wts"].astype(np.float32) for b in range(B)])
    return att, wts


# revision 17
# speedup vs baseline: 1.6791x; 1.4542x over previous
"""Bass/Tile attention kernel for Trainium2, SPMD over 8 NeuronCores.

Problem: B,S,D,DK = 8,2048,512,64 full bidirectional attention with
softmax; returns (attended[B,S,DK], weights[B,S,S]).

Sharding: data-parallel over batch — core b handles batch b. No
collectives needed. W_q/W_k/W_v replicated.

Per-core dataflow (S=2048 seq, D=512 model, DK=64 head), bf16 compute:
  x[2048,512] --cast + PE transpose--> xT[512,2048] bf16
  qT/kT[64,2048] = W.T @ xT ; vT --PE transpose--> v[2048,64]
  main loop over 16 tiles t (both paths, software-pipelined):
    weights path (t = query tile):
      S_t[128,2048] = qT_t.T @ kT   (PSUM)
      expS_t = exp(0.125*S_t)       (ACT, accum_out -> Z_t row sums)
      P_t = expS_t * (1/Z_t)        (DVE, bf16) --> DMA out (bf16 wts)
    attended path (t = key tile):
      ST_t[128,2048] = kT_t.T @ qT ; expST_t = exp(0.125*ST_t)
      PV for tile t-1 (pipelined so PE never waits on this t's exp):
        att[sq,64] += expST_{t-1}[:,sq_tile].T @ v_{t-1}
  att_t = att_t * (1/Z_t) --> DMA out (f32)
"""

import numpy as np

B, S, D, DK = 8, 2048, 512, 64
P = 128          # partition size
SQT = S // P     # 16 query tiles
C = D // P       # 4 d-chunks
U = S // P       # 16 key tiles
NCH = S // 512   # 4 free-dim 512-chunks

_CACHE = {}


def _build():
    from concourse import bacc, mybir, tile
    from concourse.masks import make_identity

    f32 = mybir.dt.float32
    bf16 = mybir.dt.bfloat16
    Exp = mybir.ActivationFunctionType.Exp

    nc = bacc.Bacc("TRN2", target_bir_lowering=False, debug=False,
                   num_devices=B)

    x_d = nc.dram_tensor("x", [S, D], f32, kind="ExternalInput").ap()
    wq_d = nc.dram_tensor("wq", [D, DK], f32, kind="ExternalInput").ap()
    wk_d = nc.dram_tensor("wk", [D, DK], f32, kind="ExternalInput").ap()
    wv_d = nc.dram_tensor("wv", [D, DK], f32, kind="ExternalInput").ap()
    att_d = nc.dram_tensor("att", [S, DK], f32, kind="ExternalOutput").ap()
    wts_d = nc.dram_tensor("wts", [S, S], bf16, kind="ExternalOutput").ap()

    with tile.TileContext(nc) as tc:
        with tc.tile_pool(name="const", bufs=1) as const:
            ident_b = const.tile([P, P], bf16)
            make_identity(nc, ident_b)

            xT = const.tile([P, C, S], bf16)
            qT = const.tile([DK, S], bf16)
            kT = const.tile([DK, S], bf16)
            vT = const.tile([DK, S], bf16)
            v_sb = const.tile([P, U * DK], bf16)
            recip = const.tile([P, SQT], f32)   # 1/Z per query tile

            # ---------------- setup phase (own PSUM pools) ----------------
            with (
                tc.tile_pool(name="xstage", bufs=1) as xstage,
                tc.tile_pool(name="trps", bufs=3, space="PSUM") as trps,
                tc.tile_pool(name="qkvps", bufs=2, space="PSUM") as qkvps,
            ):
                # weights: [512,64] f32 -> [128, 3, c, 64] bf16
                wstage = xstage.tile([P, 3, C, DK], f32, tag="wstage")
                for i, w_d in enumerate((wq_d, wk_d, wv_d)):
                    nc.sync.dma_start(
                        wstage[:, i], w_d.rearrange("(c p) k -> p c k", p=P))
                w_sb3 = const.tile([P, 3, C, DK], bf16)
                nc.vector.tensor_copy(w_sb3[:], wstage[:])

                # x: SWDGE DMA with fused f32->bf16 cast (4 chunks so
                # transposes can start early), then PE transposes
                xb = xstage.tile([P, SQT, D], bf16, tag="xb")
                for q in range(4):
                    nc.gpsimd.dma_start(
                        xb[:, 4 * q:4 * (q + 1)],
                        x_d.rearrange("(t p) d -> p t d", p=P)[
                            :, 4 * q:4 * (q + 1)])
                for t in range(SQT):
                    trp = trps.tile([P, 512], bf16, tag="tr")
                    for c in range(C):
                        nc.tensor.transpose(
                            trp[:, c * P:(c + 1) * P],
                            xb[:, t, c * P:(c + 1) * P], ident_b[:])
                    nc.vector.tensor_copy(
                        xT[:, :, t * P:(t + 1) * P],
                        trp[:].rearrange("p (c f) -> p c f", c=C))

                # qT, kT, vT = W.T @ xT
                for n in range(NCH):
                    sl = slice(n * 512, (n + 1) * 512)
                    for w_i, dst in ((0, qT), (1, kT), (2, vT)):
                        mm = qkvps.tile([DK, 512], f32, tag="qkv")
                        for c in range(C):
                            nc.tensor.matmul(
                                mm[:], w_sb3[:, w_i, c, :], xT[:, c, sl],
                                start=(c == 0), stop=(c == C - 1))
                        nc.vector.tensor_copy(dst[:, sl], mm[:])

                # v natural [128, u*64] via PE transpose of vT
                for g in range(2):
                    trp = trps.tile([P, 512], bf16, tag="tr")
                    for j in range(8):
                        u = g * 8 + j
                        nc.tensor.transpose(
                            trp[:, j * DK:(j + 1) * DK],
                            vT[:, u * P:(u + 1) * P], ident_b[:DK, :DK])
                    nc.vector.tensor_copy(
                        v_sb[:, g * 512:(g + 1) * 512], trp[:])

            # ---------------- main loop (own PSUM pools) ----------------
            with (
                tc.tile_pool(name="expp", bufs=2) as expp,
                tc.tile_pool(name="pout", bufs=3) as pout,
                tc.tile_pool(name="spsum", bufs=2, space="PSUM") as spsum,
                tc.tile_pool(name="stpsum", bufs=1, space="PSUM") as stpsum,
                tc.tile_pool(name="attpsum", bufs=1, space="PSUM") as attps,
            ):
                attp = attps.tile([P, SQT * DK], f32)
                expST_prev = None

                def pv_step(t, expST):
                    # attp is 2 PSUM banks (8 j-regions each); start/stop
                    # are bank-granular: one start (zeroes the bank) and
                    # one stop per bank per accumulation sweep.
                    for j in range(SQT):
                        nc.tensor.matmul(
                            attp[:, j * DK:(j + 1) * DK],
                            expST[:, j * P:(j + 1) * P],
                            v_sb[:, t * DK:(t + 1) * DK],
                            start=(t == 0 and j % 8 == 0),
                            stop=(t == U - 1 and j % 8 == 7))

                for t in range(SQT):
                    tsl = slice(t * P, (t + 1) * P)

                    # - weights path (sp double-buffered in halves) -
                    expS = expp.tile([P, S], bf16, tag="exp")
                    zp = expp.tile([P, 2], f32, tag="zp")
                    for h in range(2):
                        sp = spsum.tile([P, 1024], f32, tag="s")
                        for i in range(2):
                            n = h * 2 + i
                            nc.tensor.matmul(
                                sp[:, i * 512:(i + 1) * 512],
                                qT[:, tsl], kT[:, n * 512:(n + 1) * 512],
                                start=True, stop=True)
                        nc.scalar.activation(
                            expS[:, h * 1024:(h + 1) * 1024], sp[:], Exp,
                            scale=0.125, accum_out=zp[:, h:h + 1])
                    zs = expp.tile([P, 1], f32, tag="zs")
                    nc.vector.tensor_add(zs[:], zp[:, 0:1], zp[:, 1:2])
                    nc.vector.reciprocal(recip[:, t:t + 1], zs[:])
                    p_t = pout.tile([P, S], bf16, tag="p")
                    nc.vector.tensor_scalar_mul(
                        p_t[:], expS[:], recip[:, t:t + 1])
                    nc.sync.dma_start(wts_d[tsl, :], p_t[:])

                    # - attended path: ST_t, exp; PV for t-1 -
                    expST = expp.tile([P, S], bf16, tag="expst")
                    for h in range(2):
                        stp = stpsum.tile([P, 1024], f32, tag="st")
                        for i in range(2):
                            n = h * 2 + i
                            nc.tensor.matmul(
                                stp[:, i * 512:(i + 1) * 512],
                                kT[:, tsl], qT[:, n * 512:(n + 1) * 512],
                                start=True, stop=True)
                        nc.scalar.activation(
                            expST[:, h * 1024:(h + 1) * 1024], stp[:], Exp,
                            scale=0.125)
                    if expST_prev is not None:
                        pv_step(t - 1, expST_prev)
                    expST_prev = expST
                pv_step(SQT - 1, expST_prev)

                # - att = attp * recip -> out -
                for j in range(SQT):
                    a_j = pout.tile([P, DK], f32, tag="a")
                    nc.vector.tensor_scalar_mul(
                        a_j[:], attp[:, j * DK:(j + 1) * DK],
                        recip[:, j:j + 1])
                    nc.sync.dma_start(att_d[j * P:(j + 1) * P, :], a_j[:])

    nc.compile()
    return nc


def get_nc():
    if "nc" not in _CACHE:
        _CACHE["nc"] = _build()
    return _CACHE["nc"]


def kernel(inputs, W_q, W_k, W_v):
    from concourse.bass_utils import run_bass_kernel_spmd

    nc = get_nc()
    inputs = np.ascontiguousarray(inputs, dtype=np.float32)
    in_maps = [
        {
            "x": inputs[b],
            "wq": np.ascontiguousarray(W_q, dtype=np.float32),
            "wk": np.ascontiguousarray(W_k, dtype=np.float32),
            "wv": np.ascontiguousarray(W_v, dtype=np.float32),
        }
        for b in range(B)
    ]
    res = run_bass_kernel_spmd(nc, in_maps, core_ids=list(range(B)))
    att = np.stack([res.results[b]["att"] for b in range(B)])
    wts = np.stack(
        [res.results[b]["wts"].astype(np.float32) for b in range(B)])
    return att, wts
